# revision 5
# baseline (speedup 1.0000x reference)
"""Trainium2 Bass kernel for nn_ConditionalFeaturesUpsample.

Reference computation (B=1, L=64, C=80):
    x   = local_features[0].T                          # [80, 64]
    up  = ConvTranspose1d(x; wt, bt, k=stride=4)       # [80, 256]
    y   = w1 @ up + b1                                 # [3072, 256]
    out = tile(y, 75) reshaped to [128, 1, 24, 19200]  # out[ch,0,l,t] = y[l*128+ch, t%256]

Sharding: tensor-parallel over the 3072 output channels (batch is 1).
Core i computes channel rows {l*128 + 16*i + j : l in 0..23, j in 0..15},
i.e. the contiguous slice out[16*i:16*(i+1), 0, :, :] of the final output,
so the host gather is a single concatenate along axis 0.

Per-core device work: 4 tiny matmuls (ConvT), 3 matmuls [80->128, 256]
(1x1 conv; bt folded host-side into b_eff = w1 @ bt + b1), then the 75x
time-repeat is materialized in SBUF via doubling copies and streamed out
as 6 x 4.9 MB contiguous DMA writes (~30 MB/core, the HBM-write roofline).

All per-core constants ship as ONE packed [128, 771] input tensor so the
first matmul needs a single DMA-semaphore wait (PE Matmult carries at most
one sync wait in the ISA).
"""
import os
import sys

import numpy as np

for _p in ("/opt/trn_rl_repo", "/root/.axon_site/_ro/trn_rl_repo"):
    if os.path.isdir(_p) and _p not in sys.path:
        sys.path.append(_p)

import concourse.bass as bass
import concourse.bacc as bacc
import concourse.mybir as mybir
import concourse.tile as tile
from concourse.bass_utils import run_bass_kernel_spmd

UPSAMPLE_REPEAT = 75
NUM_LAYERS = 24
N_CORES = 8
GROUPS = 3             # groups of 128 channel-rows per core
T_SMALL = 256
T_FULL = T_SMALL * UPSAMPLE_REPEAT  # 19200
F32 = mybir.dt.float32

# Column layout of the packed per-core params tensor [128, P_COLS]:
#   [0:3)     beff  [128, 3]
#   [3:67)    x     [80, 64]   (rows 0:80)
#   [67:387)  wt_sb [80, 320]  (rows 0:80)
#   [387:771) w1s   [80, 384]  (rows 0:80)
C_BE, C_X, C_WT, C_W1, P_COLS = 0, 3, 67, 387, 771


def build_bass():
    nc = bacc.Bacc()
    par_d = nc.declare_dram_parameter("params", [128, P_COLS], F32, isOutput=False)
    out_d = nc.declare_dram_parameter("out", [16, NUM_LAYERS, T_FULL], F32, isOutput=True)

    with tile.TileContext(nc) as tc:
        with (
            tc.tile_pool(name="consts", bufs=1) as consts,
            tc.tile_pool(name="psum", bufs=2, space="PSUM") as psum_pool,
            tc.tile_pool(name="big", bufs=2) as big_pool,
        ):
            par_sb = consts.tile([128, P_COLS], F32)
            nc.sync.dma_start(out=par_sb[:], in_=par_d[:])
            be_sb = par_sb[:, C_BE:C_X]
            x_sb = par_sb[0:80, C_X:C_WT]
            wt_sb = par_sb[0:80, C_WT:C_W1]
            w1_sb = par_sb[0:80, C_W1:P_COLS]

            # ConvTranspose1d: psum_up[o, 64k + l] = sum_c wt_sb[c, 80k+o] x[c, l]
            up_ps = psum_pool.tile([80, T_SMALL], F32, tag="up_ps")
            for k in range(4):
                nc.tensor.matmul(
                    up_ps[:, 64 * k:64 * (k + 1)],
                    lhsT=wt_sb[:, 80 * k:80 * (k + 1)],
                    rhs=x_sb,
                    start=True,
                    stop=True,
                )
            # Rearrange [o, (k, l)] -> up[o, 4l + k] while copying PSUM -> SBUF
            up_sb = consts.tile([80, T_SMALL], F32)
            nc.vector.tensor_copy(
                out=up_sb[:].rearrange("p (l k) -> p k l", k=4),
                in_=up_ps[:].rearrange("p (k l) -> p k l", k=4),
            )

            for g in range(GROUPS):
                y_ps = psum_pool.tile([128, T_SMALL], F32, tag="y_ps")
                nc.tensor.matmul(
                    y_ps[:],
                    lhsT=w1_sb[:, 128 * g:128 * (g + 1)],
                    rhs=up_sb[:],
                    start=True,
                    stop=True,
                )
                y_big = big_pool.tile([128, T_FULL], F32, tag="y_big")
                # PSUM -> SBUF with per-partition bias add (b_eff)
                nc.scalar.activation(
                    out=y_big[:, :T_SMALL],
                    in_=y_ps[:],
                    func=mybir.ActivationFunctionType.Identity,
                    bias=be_sb[:, g:g + 1],
                )
                # Materialize the 75x repeat by doubling
                filled = T_SMALL
                while filled < T_FULL:
                    n = min(filled, T_FULL - filled)
                    nc.vector.tensor_copy(
                        out=y_big[:, filled:filled + n], in_=y_big[:, :n]
                    )
                    filled += n
                # Stream out: partition p = j*8 + l  ->  out[j, 8g + l, :].
                # Split t in half so both APs stay within 3 dims after the
                # 64KB-per-descriptor last-dim split.
                H = T_FULL // 2
                for h in range(2):
                    nc.sync.dma_start(
                        out=out_d[:, 8 * g:8 * (g + 1), h * H:(h + 1) * H],
                        in_=y_big[:, h * H:(h + 1) * H],
                    )
    nc.compile()
    return nc


def host_prep(local_features, wt, bt, w1, b1):
    lf = np.ascontiguousarray(np.asarray(local_features, np.float32))
    wt = np.asarray(wt, np.float32)
    bt = np.asarray(bt, np.float32)
    w1 = np.asarray(w1, np.float32)
    b1 = np.asarray(b1, np.float32)

    x = lf[0].T                                             # [80, 64]
    wt_sb = wt.transpose(0, 2, 1).reshape(80, 320)          # [c, 80k+o]
    b_eff = (w1.astype(np.float64) @ bt.astype(np.float64)
             + b1.astype(np.float64)).astype(np.float32)    # [3072]

    # Channel row for (core, g, p): c = (8g + p%8)*128 + 16*core + p//8
    g_idx = np.arange(GROUPS)[:, None]                      # [3, 1]
    p_idx = np.arange(128)[None, :]                         # [1, 128]
    base = (8 * g_idx + p_idx % 8) * 128 + p_idx // 8       # [3, 128]
    in_maps = []
    for core in range(N_CORES):
        c = base + 16 * core                                # [3, 128]
        params = np.zeros((128, P_COLS), np.float32)
        params[:, C_BE:C_X] = b_eff[c].T                    # [128, 3]
        params[0:80, C_X:C_WT] = x
        params[0:80, C_WT:C_W1] = wt_sb
        params[0:80, C_W1:P_COLS] = w1[c.reshape(-1), :].T  # [80, 384]
        in_maps.append({"params": params})
    return in_maps


def run(inputs, trace=False, **spmd_kwargs):
    """Returns (full_output [128,1,24,19200], BassKernelResults)."""
    nc = build_bass()
    in_maps = host_prep(**inputs)
    res = run_bass_kernel_spmd(
        nc, in_maps, core_ids=list(range(N_CORES)), trace=trace, **spmd_kwargs
    )
    shards = [np.asarray(res.results[i]["out"]) for i in range(N_CORES)]
    full = np.concatenate(shards, axis=0)          # [128, 24, 19200]
    out = full.reshape(128, 1, NUM_LAYERS, T_FULL)
    return out, res


def kernel(**inputs):
    out, _ = run(inputs, trace=False)
    return out


# revision 9
# speedup vs baseline: 1.4761x; 1.4761x over previous
"""Trainium2 Bass kernel for nn_ConditionalFeaturesUpsample.

Reference computation (B=1, L=64, C=80):
    x   = local_features[0].T                          # [80, 64]
    up  = ConvTranspose1d(x; wt, bt, k=stride=4)       # [80, 256]
    y   = w1 @ up + b1                                 # [3072, 256]
    out = tile(y, 75) reshaped to [128, 1, 24, 19200]  # out[ch,0,l,t] = y[l*128+ch, t%256]

Sharding: tensor-parallel over the 3072 output channels (batch is 1).
Core i computes channel rows {l*128 + 16*i + j : l in 0..23, j in 0..15},
i.e. the contiguous slice out[16*i:16*(i+1), 0, :, :] of the final output,
so the host gather is a single concatenate along axis 0.

Per-core device work: 4 tiny matmuls (ConvT), 3 matmuls [80->128, 256]
(1x1 conv; bt folded host-side into b_eff = w1 @ bt + b1). The 75x
time-repeat is NOT materialized: a small [128, 1280] tile (5 periods) is
built once per group and 15 chunk-DMAs per group re-read it, writing the
~30 MB/core output at ~340 GB/s (96% of the per-core HBM write wall).
Chunk size 1280 (5 KB descriptors) measured fastest: smaller chunks keep
more DMA descriptors outstanding, hiding HBM write latency.

All per-core constants ship as ONE packed [128, 771] input tensor so the
first matmul needs a single DMA-semaphore wait (PE Matmult carries at most
one sync wait in the ISA).
"""
import os
import sys

import numpy as np

for _p in ("/opt/trn_rl_repo", "/root/.axon_site/_ro/trn_rl_repo"):
    if os.path.isdir(_p) and _p not in sys.path:
        sys.path.append(_p)

import concourse.bass as bass
import concourse.bacc as bacc
import concourse.mybir as mybir
import concourse.tile as tile
from concourse.bass_utils import run_bass_kernel_spmd

UPSAMPLE_REPEAT = 75
NUM_LAYERS = 24
N_CORES = 8
GROUPS = 3             # groups of 128 channel-rows per core
T_SMALL = 256
T_FULL = T_SMALL * UPSAMPLE_REPEAT  # 19200
CHUNK = 1280           # 5 periods per chunk-DMA; measured optimum
F32 = mybir.dt.float32

# Column layout of the packed per-core params tensor [128, P_COLS]:
#   [0:3)     beff  [128, 3]
#   [3:67)    x     [80, 64]   (rows 0:80)
#   [67:387)  wt_sb [80, 320]  (rows 0:80)
#   [387:771) w1s   [80, 384]  (rows 0:80)
C_BE, C_X, C_WT, C_W1, P_COLS = 0, 3, 67, 387, 771


def build_bass():
    nc = bacc.Bacc()
    par_d = nc.declare_dram_parameter("params", [128, P_COLS], F32, isOutput=False)
    out_d = nc.declare_dram_parameter("out", [16, NUM_LAYERS, T_FULL], F32, isOutput=True)

    with tile.TileContext(nc) as tc:
        with (
            tc.tile_pool(name="consts", bufs=1) as consts,
            tc.tile_pool(name="psum", bufs=2, space="PSUM") as psum_pool,
            tc.tile_pool(name="mid", bufs=3) as mid_pool,
        ):
            par_sb = consts.tile([128, P_COLS], F32)
            nc.sync.dma_start(out=par_sb[:], in_=par_d[:])
            be_sb = par_sb[:, C_BE:C_X]
            x_sb = par_sb[0:80, C_X:C_WT]
            wt_sb = par_sb[0:80, C_WT:C_W1]
            w1_sb = par_sb[0:80, C_W1:P_COLS]

            # ConvTranspose1d: psum_up[o, 64k + l] = sum_c wt_sb[c, 80k+o] x[c, l]
            up_ps = psum_pool.tile([80, T_SMALL], F32, tag="up_ps")
            for k in range(4):
                nc.tensor.matmul(
                    up_ps[:, 64 * k:64 * (k + 1)],
                    lhsT=wt_sb[:, 80 * k:80 * (k + 1)],
                    rhs=x_sb,
                    start=True,
                    stop=True,
                )
            # Rearrange [o, (k, l)] -> up[o, 4l + k] while copying PSUM -> SBUF
            up_sb = consts.tile([80, T_SMALL], F32)
            nc.vector.tensor_copy(
                out=up_sb[:].rearrange("p (l k) -> p k l", k=4),
                in_=up_ps[:].rearrange("p (k l) -> p k l", k=4),
            )

            for g in range(GROUPS):
                y_ps = psum_pool.tile([128, T_SMALL], F32, tag="y_ps")
                nc.tensor.matmul(
                    y_ps[:],
                    lhsT=w1_sb[:, 128 * g:128 * (g + 1)],
                    rhs=up_sb[:],
                    start=True,
                    stop=True,
                )
                y_mid = mid_pool.tile([128, CHUNK], F32, tag="y_mid")
                # PSUM -> SBUF with per-partition bias add (b_eff)
                nc.scalar.activation(
                    out=y_mid[:, :T_SMALL],
                    in_=y_ps[:],
                    func=mybir.ActivationFunctionType.Identity,
                    bias=be_sb[:, g:g + 1],
                )
                # Fill the rest of the 5-period tile by doubling
                filled = T_SMALL
                while filled < CHUNK:
                    n = min(filled, CHUNK - filled)
                    nc.vector.tensor_copy(
                        out=y_mid[:, filled:filled + n], in_=y_mid[:, :n]
                    )
                    filled += n
                # Stream out: partition p = j*8 + l  ->  out[j, 8g + l, :],
                # 15 chunk DMAs re-reading the same 5-period tile.
                for h in range(T_FULL // CHUNK):
                    nc.sync.dma_start(
                        out=out_d[:, 8 * g:8 * (g + 1),
                                  h * CHUNK:(h + 1) * CHUNK],
                        in_=y_mid[:],
                    )
    nc.compile()
    return nc


def host_prep(local_features, wt, bt, w1, b1):
    lf = np.ascontiguousarray(np.asarray(local_features, np.float32))
    wt = np.asarray(wt, np.float32)
    bt = np.asarray(bt, np.float32)
    w1 = np.asarray(w1, np.float32)
    b1 = np.asarray(b1, np.float32)

    x = lf[0].T                                             # [80, 64]
    wt_sb = wt.transpose(0, 2, 1).reshape(80, 320)          # [c, 80k+o]
    b_eff = (w1.astype(np.float64) @ bt.astype(np.float64)
             + b1.astype(np.float64)).astype(np.float32)    # [3072]

    # Channel row for (core, g, p): c = (8g + p%8)*128 + 16*core + p//8
    g_idx = np.arange(GROUPS)[:, None]                      # [3, 1]
    p_idx = np.arange(128)[None, :]                         # [1, 128]
    base = (8 * g_idx + p_idx % 8) * 128 + p_idx // 8       # [3, 128]
    in_maps = []
    for core in range(N_CORES):
        c = base + 16 * core                                # [3, 128]
        params = np.zeros((128, P_COLS), np.float32)
        params[:, C_BE:C_X] = b_eff[c].T                    # [128, 3]
        params[0:80, C_X:C_WT] = x
        params[0:80, C_WT:C_W1] = wt_sb
        params[0:80, C_W1:P_COLS] = w1[c.reshape(-1), :].T  # [80, 384]
        in_maps.append({"params": params})
    return in_maps


def run(inputs, trace=False, **spmd_kwargs):
    """Returns (full_output [128,1,24,19200], BassKernelResults)."""
    nc = build_bass()
    in_maps = host_prep(**inputs)
    res = run_bass_kernel_spmd(
        nc, in_maps, core_ids=list(range(N_CORES)), trace=trace, **spmd_kwargs
    )
    shards = [np.asarray(res.results[i]["out"]) for i in range(N_CORES)]
    full = np.concatenate(shards, axis=0)          # [128, 24, 19200]
    out = full.reshape(128, 1, NUM_LAYERS, T_FULL)
    return out, res


def kernel(**inputs):
    out, _ = run(inputs, trace=False)
    return out
